# revision 8
# baseline (speedup 1.0000x reference)
"""Fused sp2norm-MHA kernel for Trainium2, 8 NeuronCores.

Model (per reference):
    qkv = x @ W_qkv.T ; split heads (H=16, hs=64)
    s = (q @ k.T) / sqrt(hs);  w = softplus(s) causal-masked
    out_h = (w @ v) / ||w||_row ;  out = concat(out_h) @ W_proj.T + b_proj

Sharding: core c = (b, g) with b = c // 4 (batch), g = c % 4 (head group of 4).
Each core computes its batch's QKV for its 4 heads, the attention, and a
partial projection over its 256 feature channels. The host sums the 4 partial
projections per batch and adds the bias (the unshard step).

On-device layout (per core):
    xT  [1024, 2048]  bf16  = x[b].T                  (c on partitions)
    Sᵀ attention: scores computed transposed [j, i] (keys on partitions) so
    softplus output w feeds (a) out.T = V.T-style matmul lhsT=V[j,d], and
    (b) norm² = ones.T @ w² — both contract over j on partitions.
    softplus = Exp (scale=1/8, PSUM src) then Ln (bias=1.0), fp16 intermediate.
    Causality: block-ragged i-range per j-block + a -1e9 mask matmul on the
    diagonal 128x128 sub-block (softplus underflows to exactly 0).
"""

import numpy as np
import ml_dtypes

import concourse.bacc as bacc
import concourse.tile as tile
import concourse.mybir as mybir
from concourse.bass_utils import run_bass_kernel_spmd

dt = mybir.dt
F32, F32R, F16, BF16 = dt.float32, dt.float32r, dt.float16, dt.bfloat16
AF = mybir.ActivationFunctionType

B, T, C, H, HS = 2, 2048, 1024, 16, 64
HPC = 4            # heads per core
NCORES = 8
SCALE = 1.0 / np.sqrt(HS)
TC_Q = 512         # i-chunk (query) width
JB = 128           # j-block (key) width

_CACHE = {}


def _build():
    nc = bacc.Bacc(None, target_bir_lowering=False)

    xT = nc.dram_tensor("xT", [C, T], BF16, kind="ExternalInput")
    wqk = nc.dram_tensor("wqk", [C, 512], BF16, kind="ExternalInput")
    wv = nc.dram_tensor("wv", [C, 256], BF16, kind="ExternalInput")
    wp = nc.dram_tensor("wp", [256, C], BF16, kind="ExternalInput")
    mT = nc.dram_tensor("mT", [128, 128], BF16, kind="ExternalInput")
    idn = nc.dram_tensor("idn", [128, 128], BF16, kind="ExternalInput")
    out = nc.dram_tensor("out", [T, C], F32, kind="ExternalOutput")

    with tile.TileContext(nc) as tc:
        with (
            tc.tile_pool(name="cst", bufs=1) as cst,
            tc.tile_pool(name="data", bufs=1) as data,
        ):
            # ---- constants / weights ----
            wqk_sb = cst.tile([128, 8, 512], BF16)
            nc.sync.dma_start(wqk_sb, wqk[:].rearrange("(po pi) j -> pi po j", pi=128))
            wv_sb = cst.tile([128, 8, 256], BF16)
            nc.sync.dma_start(wv_sb, wv[:].rearrange("(po pi) j -> pi po j", pi=128))
            wp_sb = cst.tile([128, 2, 1024], BF16)
            nc.sync.dma_start(wp_sb, wp[:].rearrange("(po pi) e -> pi po e", pi=128))
            mT_sb = cst.tile([128, 128], BF16)
            nc.sync.dma_start(mT_sb, mT[:])
            idn_sb = cst.tile([128, 128], BF16)
            nc.sync.dma_start(idn_sb, idn[:])
            ones_n = cst.tile([128, 1], BF16)
            nc.vector.memset(ones_n, 1.0)
            ones_b32 = cst.tile([128, 64], F32)
            nc.vector.memset(ones_b32, 1.0)

            # ---- x.T resident (4MB bf16), per c-block DMAs so QKV starts early
            xT_sb = data.tile([128, 8, 2048], BF16)
            xT_r = xT[:].rearrange("(po pi) t -> pi po t", pi=128)
            for cb in range(8):
                nc.sync.dma_start(xT_sb[:, cb, :], xT_r[:, cb, :])

            # qkT: block 0,1 = q head-pairs; block 2,3 = k head-pairs.
            # Partition rows (h%2)*64..+64 inside each block = one head.
            qkT = data.tile([128, 4, 2048], BF16)
            v_sb = data.tile([128, 16, 256], BF16)
            yT = data.tile([128, 2, 2048], BF16)

            # ================= phase 1: QKV projections =================
            with tc.tile_pool(name="ps_qkv", bufs=2, space="PSUM") as ps_qkv:
                for jb in range(4):
                    for tcc in range(4):
                        pq = ps_qkv.tile([128, 512], F32, tag="qk")
                        for cb in range(8):
                            nc.tensor.matmul(
                                pq,
                                wqk_sb[:, cb, jb * 128:(jb + 1) * 128],
                                xT_sb[:, cb, tcc * 512:(tcc + 1) * 512],
                                start=(cb == 0), stop=(cb == 7),
                            )
                        nc.vector.tensor_copy(
                            qkT[:, jb, tcc * 512:(tcc + 1) * 512], pq)
                for tb in range(16):
                    pv = ps_qkv.tile([128, 256], F32, tag="v")
                    for cb in range(8):
                        nc.tensor.matmul(
                            pv,
                            xT_sb[:, cb, tb * 128:(tb + 1) * 128],
                            wv_sb[:, cb, :],
                            start=(cb == 0), stop=(cb == 7),
                        )
                    nc.vector.tensor_copy(v_sb[:, tb, :], pv)

            # ================= phase 2: attention (2 head pairs) =================
            with (
                tc.tile_pool(name="ps_s", bufs=2, space="PSUM") as ps_s,   # 4 banks
                tc.tile_pool(name="ps_o", bufs=2, space="PSUM") as ps_o,   # 2 banks
                tc.tile_pool(name="ps_n", bufs=1, space="PSUM") as ps_n,   # 1 bank
                tc.tile_pool(name="ps_b", bufs=1, space="PSUM") as ps_b,   # 1 bank
                tc.tile_pool(name="we", bufs=3) as we,
                tc.tile_pool(name="epi", bufs=2) as epi,
            ):
                for hp in range(2):
                    qblk, kblk = hp, 2 + hp
                    for ic in range(4):
                        po = ps_o.tile([128, 512], F32, tag="po")
                        pn = ps_n.tile([128, 512], F32, tag="pn")
                        njb = 4 * ic + 4
                        for jb in range(njb):
                            m = jb - 4 * ic
                            N = 512 if m < 0 else 512 - 128 * m
                            ioff = ic * 512 + (512 - N)
                            ps_ = ps_s.tile([128, 1024], F32, tag="ps")
                            diag = m >= 0
                            # scores (transposed): K=64, two heads row-packed
                            nc.tensor.matmul(
                                ps_[:, 0:N],
                                qkT[0:64, kblk, jb * 128:(jb + 1) * 128],
                                qkT[0:64, qblk, ioff:ioff + N],
                                start=True, stop=not diag,
                            )
                            nc.tensor.matmul(
                                ps_[:, 512:512 + N],
                                qkT[64:128, kblk, jb * 128:(jb + 1) * 128],
                                qkT[64:128, qblk, ioff:ioff + N],
                                start=True, stop=not diag,
                            )
                            if diag:
                                nc.tensor.matmul(ps_[:, 0:128], mT_sb, idn_sb,
                                                 start=False, stop=True)
                                nc.tensor.matmul(ps_[:, 512:640], mT_sb, idn_sb,
                                                 start=False, stop=True)
                            # softplus = Ln(Exp(s/8) + 1), fp16 intermediate
                            e = we.tile([128, 1024], F16, tag="e")
                            w = we.tile([128, 1024], BF16, tag="w")
                            w2 = we.tile([128, 1024], BF16, tag="w2")
                            if N == 512:
                                nc.scalar.activation(e, ps_, AF.Exp, scale=SCALE)
                                nc.scalar.activation(w, e, AF.Ln, bias=1.0)
                                nc.vector.tensor_mul(w2, w, w)
                            else:
                                nc.scalar.activation(e[:, 0:N], ps_[:, 0:N],
                                                     AF.Exp, scale=SCALE)
                                nc.scalar.activation(e[:, 512:512 + N],
                                                     ps_[:, 512:512 + N],
                                                     AF.Exp, scale=SCALE)
                                nc.scalar.activation(w[:, 0:N], e[:, 0:N],
                                                     AF.Ln, bias=1.0)
                                nc.scalar.activation(w[:, 512:512 + N],
                                                     e[:, 512:512 + N],
                                                     AF.Ln, bias=1.0)
                                nc.vector.tensor_mul(w2[:, 0:N], w[:, 0:N],
                                                     w[:, 0:N])
                                nc.vector.tensor_mul(w2[:, 512:512 + N],
                                                     w[:, 512:512 + N],
                                                     w[:, 512:512 + N])
                            # out.T accumulation: lhsT = V[j, d], col-packed heads
                            last = jb == njb - 1
                            hA, hB = 2 * hp, 2 * hp + 1
                            nc.tensor.matmul(
                                po[0:64, 512 - N:512],
                                v_sb[:, jb, hA * 64:hA * 64 + 64],
                                w[:, 0:N],
                                start=(jb == 0), stop=last, tile_position=(0, 0),
                            )
                            nc.tensor.matmul(
                                po[64:128, 512 - N:512],
                                v_sb[:, jb, hB * 64:hB * 64 + 64],
                                w[:, 512:512 + N],
                                start=(jb == 0), stop=last, tile_position=(0, 64),
                            )
                            # norm^2 accumulation: ones.T @ w2
                            nc.tensor.matmul(
                                pn[0:1, 512 - N:512], ones_n, w2[:, 0:N],
                                start=(jb == 0), stop=last, tile_position=(0, 0),
                            )
                            nc.tensor.matmul(
                                pn[64:65, 512 - N:512], ones_n,
                                w2[:, 512:512 + N],
                                start=(jb == 0), stop=last, tile_position=(0, 64),
                            )
                        # ---- chunk epilogue: y = out.T * rsqrt(norm2) ----
                        # rsqrt = Exp(-0.5 * Ln(x)): stays in the Exp/Ln
                        # activation-table set (no ACT_TABLE_LOAD thrash).
                        nrm = epi.tile([128, 512], F32, tag="nrm")
                        nc.scalar.activation(nrm, pn, AF.Ln)
                        rs = epi.tile([128, 512], F32, tag="rs")
                        nc.scalar.activation(rs, nrm, AF.Exp, scale=-0.5)
                        pb = ps_b.tile([128, 512], F32, tag="pb")
                        nc.tensor.matmul(pb[0:64, :], ones_b32[0:1, :],
                                         rs[0:1, :],
                                         start=True, stop=True,
                                         tile_position=(0, 0))
                        nc.tensor.matmul(pb[64:128, :], ones_b32[64:65, :],
                                         rs[64:65, :],
                                         start=True, stop=True,
                                         tile_position=(64, 64))
                        rb = epi.tile([128, 512], F32, tag="rb")
                        nc.vector.tensor_copy(rb, pb)
                        nc.vector.tensor_mul(
                            yT[:, hp, ic * 512:(ic + 1) * 512], po, rb)

            # ================= phase 3: partial projection =================
            with (
                tc.tile_pool(name="ps_p", bufs=2, space="PSUM") as ps_p,
                tc.tile_pool(name="outp", bufs=3) as outp,
            ):
                for tcc in range(16):
                    pp = ps_p.tile([128, 1024], F32, tag="pp")
                    for kb in range(2):
                        for nk in range(2):
                            nc.tensor.matmul(
                                pp[:, nk * 512:(nk + 1) * 512],
                                yT[:, kb, tcc * 128:(tcc + 1) * 128],
                                wp_sb[:, kb, nk * 512:(nk + 1) * 512],
                                start=(kb == 0), stop=(kb == 1),
                            )
                    os_ = outp.tile([128, 1024], F32, tag="os")
                    nc.vector.tensor_copy(os_, pp)
                    nc.sync.dma_start(out[tcc * 128:(tcc + 1) * 128, :], os_)

    nc.compile()
    return nc


def _prep_inputs(x, W_qkv, W_proj):
    """Host-side shard + layout prep. Returns per-core input maps."""
    bf = ml_dtypes.bfloat16
    maskT = np.triu(np.full((128, 128), -1e9, dtype=np.float32), 1).astype(bf)
    ident = np.eye(128, dtype=np.float32).astype(bf)
    in_maps = []
    for core in range(NCORES):
        b, g = core // 4, core % 4
        heads = range(4 * g, 4 * g + 4)
        # W_qkv rows: q = h*64.., k = C + h*64.., v = 2C + h*64..
        q_rows = np.concatenate([np.arange(h * HS, (h + 1) * HS) for h in heads])
        wqk = np.concatenate(
            [W_qkv[q_rows, :].T, W_qkv[C + q_rows, :].T], axis=1)  # [C, 512]
        wv = W_qkv[2 * C + q_rows, :].T                            # [C, 256]
        wp = W_proj[:, q_rows].T                                   # [256, C]
        in_maps.append({
            "xT": np.ascontiguousarray(x[b].T).astype(bf),
            "wqk": np.ascontiguousarray(wqk).astype(bf),
            "wv": np.ascontiguousarray(wv).astype(bf),
            "wp": np.ascontiguousarray(wp).astype(bf),
            "mT": maskT,
            "idn": ident,
        })
    return in_maps


def _run(in_maps, trace=False, trace_cores=None):
    if "nc" not in _CACHE:
        _CACHE["nc"] = _build()
    return run_bass_kernel_spmd(
        _CACHE["nc"], in_maps, core_ids=list(range(NCORES)),
        trace=trace, trace_cores=trace_cores,
    )


def kernel(x, W_qkv, W_proj, b_proj):
    x = np.asarray(x, dtype=np.float32)
    W_qkv = np.asarray(W_qkv, dtype=np.float32)
    W_proj = np.asarray(W_proj, dtype=np.float32)
    b_proj = np.asarray(b_proj, dtype=np.float32)

    res = _run(_prep_inputs(x, W_qkv, W_proj)).results
    out = np.zeros((B, T, C), dtype=np.float64)
    for core in range(NCORES):
        out[core // 4] += np.asarray(res[core]["out"], dtype=np.float64)
    out += b_proj.astype(np.float64)
    return out.astype(np.float32)


# revision 10
# speedup vs baseline: 1.5406x; 1.5406x over previous
"""Fused sp2norm-MHA kernel for Trainium2, 8 NeuronCores.

Model (per reference):
    qkv = x @ W_qkv.T ; split heads (H=16, hs=64)
    s = (q @ k.T) / sqrt(hs);  w = softplus(s) causal-masked
    out_h = (w @ v) / ||w||_row ;  out = concat(out_h) @ W_proj.T + b_proj

Sharding: core c = (b, g) with b = c // 4 (batch), g = c % 4 (head group of 4).
Each core computes its batch's QKV for its 4 heads, the attention, and a
partial projection over its 256 feature channels. The host sums the 4 partial
projections per batch and adds the bias (the unshard step).

On-device layout (per core):
    xT  [1024, 2048]  bf16  = x[b].T                  (c on partitions)
    Sᵀ attention: scores computed transposed [j, i] (keys on partitions) so
    softplus output w feeds (a) out.T = V.T-style matmul lhsT=V[j,d], and
    (b) norm² = ones.T @ w² — both contract over j on partitions.
    softplus = Exp (scale=1/8, PSUM src) then Ln (bias=1.0), fp16 intermediate.
    Causality: block-ragged i-range per j-block + a -1e9 mask matmul on the
    diagonal 128x128 sub-block (softplus underflows to exactly 0).
"""

import numpy as np
import ml_dtypes

import concourse.bacc as bacc
import concourse.tile as tile
import concourse.mybir as mybir
from concourse.bass_utils import run_bass_kernel_spmd

# The act-table-set chooser assigns each activation the FIRST set containing
# its function; with the default ordering Exp -> exp_and_others and
# Ln -> natural_log, so alternating Exp/Ln thrashes ACT_TABLE_LOAD (~1.3us
# each, >100 loads). Reorder so the combined Exp+Ln set is preferred.
_orig_get_tables = bacc.get_activation_tables


def _tables_ln_exp_first(arch):
    t = _orig_get_tables(arch)
    key = "natural_log_exp_and_others"
    if key not in t:
        return t
    # Keep dict ORDER (set ids are positional); drop Exp/Ln from every other
    # set so the combined set is the unique candidate for both.
    exp = mybir.ActivationFunctionType.Exp
    ln = mybir.ActivationFunctionType.Ln
    out = {}
    for k, fns in t.items():
        out[k] = fns if k == key else (set(fns) - {exp, ln})
    return out


bacc.get_activation_tables = _tables_ln_exp_first

dt = mybir.dt
F32, F32R, F16, BF16 = dt.float32, dt.float32r, dt.float16, dt.bfloat16
AF = mybir.ActivationFunctionType

B, T, C, H, HS = 2, 2048, 1024, 16, 64
HPC = 4            # heads per core
NCORES = 8
SCALE = 1.0 / np.sqrt(HS)
TC_Q = 512         # i-chunk (query) width
JB = 128           # j-block (key) width

_CACHE = {}


def _build():
    nc = bacc.Bacc(None, target_bir_lowering=False)

    xT = nc.dram_tensor("xT", [C, T], BF16, kind="ExternalInput")
    wqk = nc.dram_tensor("wqk", [C, 512], BF16, kind="ExternalInput")
    wv = nc.dram_tensor("wv", [C, 256], BF16, kind="ExternalInput")
    wp = nc.dram_tensor("wp", [256, C], BF16, kind="ExternalInput")
    mT = nc.dram_tensor("mT", [128, 128], BF16, kind="ExternalInput")
    idn = nc.dram_tensor("idn", [128, 128], BF16, kind="ExternalInput")
    out = nc.dram_tensor("out", [T, C], F32, kind="ExternalOutput")

    with tile.TileContext(nc) as tc:
        with (
            tc.tile_pool(name="cst", bufs=1) as cst,
            tc.tile_pool(name="data", bufs=1) as data,
        ):
            # ---- constants / weights ----
            wqk_sb = cst.tile([128, 8, 512], BF16)
            nc.sync.dma_start(wqk_sb, wqk[:].rearrange("(po pi) j -> pi po j", pi=128))
            wv_sb = cst.tile([128, 8, 256], BF16)
            nc.sync.dma_start(wv_sb, wv[:].rearrange("(po pi) j -> pi po j", pi=128))
            wp_sb = cst.tile([128, 2, 1024], BF16)
            nc.sync.dma_start(wp_sb, wp[:].rearrange("(po pi) e -> pi po e", pi=128))
            mT_sb = cst.tile([128, 128], BF16)
            nc.sync.dma_start(mT_sb, mT[:])
            idn_sb = cst.tile([128, 128], BF16)
            nc.sync.dma_start(idn_sb, idn[:])
            ones_n = cst.tile([128, 1], BF16)
            nc.vector.memset(ones_n, 1.0)
            ones_b32 = cst.tile([128, 64], F32)
            nc.vector.memset(ones_b32, 1.0)

            # ---- x.T resident (4MB bf16), per c-block DMAs so QKV starts early
            xT_sb = data.tile([128, 8, 2048], BF16)
            xT_r = xT[:].rearrange("(po pi) t -> pi po t", pi=128)
            for cb in range(8):
                nc.sync.dma_start(xT_sb[:, cb, :], xT_r[:, cb, :])

            # qkT: block 0,1 = q head-pairs; block 2,3 = k head-pairs.
            # Partition rows (h%2)*64..+64 inside each block = one head.
            qkT = data.tile([128, 4, 2048], BF16)
            v_sb = data.tile([128, 16, 256], BF16)
            yT = data.tile([128, 2, 2048], BF16)

            # ================= phase 1: QKV projections =================
            with tc.tile_pool(name="ps_qkv", bufs=2, space="PSUM") as ps_qkv:
                for jb in range(4):
                    for tcc in range(4):
                        pq = ps_qkv.tile([128, 512], F32, tag="qk")
                        for cb in range(8):
                            nc.tensor.matmul(
                                pq,
                                wqk_sb[:, cb, jb * 128:(jb + 1) * 128],
                                xT_sb[:, cb, tcc * 512:(tcc + 1) * 512],
                                start=(cb == 0), stop=(cb == 7),
                            )
                        nc.vector.tensor_copy(
                            qkT[:, jb, tcc * 512:(tcc + 1) * 512], pq)
                for tb in range(16):
                    pv = ps_qkv.tile([128, 256], F32, tag="v")
                    for cb in range(8):
                        nc.tensor.matmul(
                            pv,
                            xT_sb[:, cb, tb * 128:(tb + 1) * 128],
                            wv_sb[:, cb, :],
                            start=(cb == 0), stop=(cb == 7),
                        )
                    nc.vector.tensor_copy(v_sb[:, tb, :], pv)

            # ================= phase 2: attention (2 head pairs) =================
            with (
                tc.tile_pool(name="ps_s", bufs=2, space="PSUM") as ps_s,   # 4 banks
                tc.tile_pool(name="ps_o", bufs=2, space="PSUM") as ps_o,   # 2 banks
                tc.tile_pool(name="ps_n", bufs=1, space="PSUM") as ps_n,   # 1 bank
                tc.tile_pool(name="ps_b", bufs=1, space="PSUM") as ps_b,   # 1 bank
                tc.tile_pool(name="we", bufs=3) as we,
                tc.tile_pool(name="epi", bufs=2) as epi,
            ):
                for hp in range(2):
                    qblk, kblk = hp, 2 + hp
                    for ic in range(4):
                        po = ps_o.tile([128, 512], F32, tag="po")
                        pn = ps_n.tile([128, 512], F32, tag="pn")
                        njb = 4 * ic + 4
                        for jb in range(njb):
                            m = jb - 4 * ic
                            N = 512 if m < 0 else 512 - 128 * m
                            ioff = ic * 512 + (512 - N)
                            ps_ = ps_s.tile([128, 1024], F32, tag="ps")
                            diag = m >= 0
                            # scores (transposed): K=64, two heads row-packed
                            nc.tensor.matmul(
                                ps_[:, 0:N],
                                qkT[0:64, kblk, jb * 128:(jb + 1) * 128],
                                qkT[0:64, qblk, ioff:ioff + N],
                                start=True, stop=not diag,
                            )
                            nc.tensor.matmul(
                                ps_[:, 512:512 + N],
                                qkT[64:128, kblk, jb * 128:(jb + 1) * 128],
                                qkT[64:128, qblk, ioff:ioff + N],
                                start=True, stop=not diag,
                            )
                            if diag:
                                nc.tensor.matmul(ps_[:, 0:128], mT_sb, idn_sb,
                                                 start=False, stop=True)
                                nc.tensor.matmul(ps_[:, 512:640], mT_sb, idn_sb,
                                                 start=False, stop=True)
                            # softplus = Ln(Exp(s/8) + 1), fp16 intermediate
                            e = we.tile([128, 1024], F16, tag="e")
                            w = we.tile([128, 1024], BF16, tag="w")
                            w2 = we.tile([128, 1024], BF16, tag="w2")
                            if N == 512:
                                nc.scalar.activation(e, ps_, AF.Exp, scale=SCALE)
                                nc.scalar.activation(w, e, AF.Ln, bias=1.0)
                                nc.vector.tensor_mul(w2, w, w)
                            else:
                                nc.scalar.activation(e[:, 0:N], ps_[:, 0:N],
                                                     AF.Exp, scale=SCALE)
                                nc.scalar.activation(e[:, 512:512 + N],
                                                     ps_[:, 512:512 + N],
                                                     AF.Exp, scale=SCALE)
                                nc.scalar.activation(w[:, 0:N], e[:, 0:N],
                                                     AF.Ln, bias=1.0)
                                nc.scalar.activation(w[:, 512:512 + N],
                                                     e[:, 512:512 + N],
                                                     AF.Ln, bias=1.0)
                                nc.vector.tensor_mul(w2[:, 0:N], w[:, 0:N],
                                                     w[:, 0:N])
                                nc.vector.tensor_mul(w2[:, 512:512 + N],
                                                     w[:, 512:512 + N],
                                                     w[:, 512:512 + N])
                            # out.T accumulation: lhsT = V[j, d], col-packed heads
                            last = jb == njb - 1
                            hA, hB = 2 * hp, 2 * hp + 1
                            nc.tensor.matmul(
                                po[0:64, 512 - N:512],
                                v_sb[:, jb, hA * 64:hA * 64 + 64],
                                w[:, 0:N],
                                start=(jb == 0), stop=last, tile_position=(0, 0),
                            )
                            nc.tensor.matmul(
                                po[64:128, 512 - N:512],
                                v_sb[:, jb, hB * 64:hB * 64 + 64],
                                w[:, 512:512 + N],
                                start=(jb == 0), stop=last, tile_position=(0, 64),
                            )
                            # norm^2 accumulation: ones.T @ w2
                            nc.tensor.matmul(
                                pn[0:1, 512 - N:512], ones_n, w2[:, 0:N],
                                start=(jb == 0), stop=last, tile_position=(0, 0),
                            )
                            nc.tensor.matmul(
                                pn[64:65, 512 - N:512], ones_n,
                                w2[:, 512:512 + N],
                                start=(jb == 0), stop=last, tile_position=(0, 64),
                            )
                        # ---- chunk epilogue: y = out.T * rsqrt(norm2) ----
                        # rsqrt = Exp(-0.5 * Ln(x)): stays in the Exp/Ln
                        # activation-table set (no ACT_TABLE_LOAD thrash).
                        nrm = epi.tile([128, 512], F32, tag="nrm")
                        nc.scalar.activation(nrm, pn, AF.Ln)
                        rs = epi.tile([128, 512], F32, tag="rs")
                        nc.scalar.activation(rs, nrm, AF.Exp, scale=-0.5)
                        pb = ps_b.tile([128, 512], F32, tag="pb")
                        nc.tensor.matmul(pb[0:64, :], ones_b32[0:1, :],
                                         rs[0:1, :],
                                         start=True, stop=True,
                                         tile_position=(0, 0))
                        nc.tensor.matmul(pb[64:128, :], ones_b32[64:65, :],
                                         rs[64:65, :],
                                         start=True, stop=True,
                                         tile_position=(64, 64))
                        rb = epi.tile([128, 512], F32, tag="rb")
                        nc.vector.tensor_copy(rb, pb)
                        nc.vector.tensor_mul(
                            yT[:, hp, ic * 512:(ic + 1) * 512], po, rb)

            # ================= phase 3: partial projection =================
            with (
                tc.tile_pool(name="ps_p", bufs=2, space="PSUM") as ps_p,
                tc.tile_pool(name="outp", bufs=3) as outp,
            ):
                for tcc in range(16):
                    pp = ps_p.tile([128, 1024], F32, tag="pp")
                    for kb in range(2):
                        for nk in range(2):
                            nc.tensor.matmul(
                                pp[:, nk * 512:(nk + 1) * 512],
                                yT[:, kb, tcc * 128:(tcc + 1) * 128],
                                wp_sb[:, kb, nk * 512:(nk + 1) * 512],
                                start=(kb == 0), stop=(kb == 1),
                            )
                    os_ = outp.tile([128, 1024], F32, tag="os")
                    nc.vector.tensor_copy(os_, pp)
                    nc.sync.dma_start(out[tcc * 128:(tcc + 1) * 128, :], os_)

    nc.compile()
    return nc


def _prep_inputs(x, W_qkv, W_proj):
    """Host-side shard + layout prep. Returns per-core input maps."""
    bf = ml_dtypes.bfloat16
    maskT = np.triu(np.full((128, 128), -1e9, dtype=np.float32), 1).astype(bf)
    ident = np.eye(128, dtype=np.float32).astype(bf)
    in_maps = []
    for core in range(NCORES):
        b, g = core // 4, core % 4
        heads = range(4 * g, 4 * g + 4)
        # W_qkv rows: q = h*64.., k = C + h*64.., v = 2C + h*64..
        q_rows = np.concatenate([np.arange(h * HS, (h + 1) * HS) for h in heads])
        wqk = np.concatenate(
            [W_qkv[q_rows, :].T, W_qkv[C + q_rows, :].T], axis=1)  # [C, 512]
        wv = W_qkv[2 * C + q_rows, :].T                            # [C, 256]
        wp = W_proj[:, q_rows].T                                   # [256, C]
        in_maps.append({
            "xT": np.ascontiguousarray(x[b].T).astype(bf),
            "wqk": np.ascontiguousarray(wqk).astype(bf),
            "wv": np.ascontiguousarray(wv).astype(bf),
            "wp": np.ascontiguousarray(wp).astype(bf),
            "mT": maskT,
            "idn": ident,
        })
    return in_maps


def _run(in_maps, trace=False, trace_cores=None):
    if "nc" not in _CACHE:
        _CACHE["nc"] = _build()
    return run_bass_kernel_spmd(
        _CACHE["nc"], in_maps, core_ids=list(range(NCORES)),
        trace=trace, trace_cores=trace_cores,
    )


def kernel(x, W_qkv, W_proj, b_proj):
    x = np.asarray(x, dtype=np.float32)
    W_qkv = np.asarray(W_qkv, dtype=np.float32)
    W_proj = np.asarray(W_proj, dtype=np.float32)
    b_proj = np.asarray(b_proj, dtype=np.float32)

    res = _run(_prep_inputs(x, W_qkv, W_proj)).results
    out = np.zeros((B, T, C), dtype=np.float64)
    for core in range(NCORES):
        out[core // 4] += np.asarray(res[core]["out"], dtype=np.float64)
    out += b_proj.astype(np.float64)
    return out.astype(np.float32)
